# revision 1
# baseline (speedup 1.0000x reference)
"""Trainium2 Bass kernel: GQA decode attention (sparse / masked KV cache).

Module: q/k/v projections (+bias) -> RoPE(q, k_new) -> ring-buffer cache
insert at end_index -> masked softmax attention over the KV cache -> output
projection.

Sharding (8 NeuronCores): tp=4 over kv-heads (each tp rank owns 1 kv head and
its 7 GQA q heads, wq/wo/q_bias sharded on the head axis) x fsdp=2 over batch
(16 batches per rank) for cache/activations.  Each core computes a partial
output (its 7 heads' contribution to out[b, :]); the 4-way tp partial sum is
done host-side after gathering.

Device algorithm per core, per batch:
  scores^T[l, q] accumulated per 128-slot chunk via
      matmul(lhsT=KT_chunk[h,128l], rhs=QT_b[h,7])             (PSUM [128,7])
  P^T = exp(scale * scores^T + maskbias)                        (ScalarE, no
      max-subtraction: scores are bounded <<88 so fp32 exp is exact-safe)
  O^T[h, q]  += matmul(lhsT=V_chunk[l,h],  rhs=P^T_chunk[l,7])  (PSUM accum)
  denom[1,q] = matmul(lhsT=ones[l,1], rhs=P^T_all) + free-axis reduce
  QKV^T = O^T * broadcast(1/denom)
  out[b,:]  += matmul(lhsT=QKV^T[:, q, :], rhs=WO_q[h, d]) over q

K arrives host-pre-transposed per (batch, kv-head) as [h, l] so scores need no
on-device transpose; V stays in natural [l, h] chunk layout.  The new-token
k/v are projected on device, RoPE'd, and inserted into the SBUF-resident
chunk that holds slot end_index before use.
"""

import numpy as np

import concourse.bass as bass
import concourse.mybir as mybir
import concourse.tile as tile
from concourse import bacc
from concourse.bass_utils import run_bass_kernel_spmd
from concourse.masks import make_identity

# Problem shapes (hardcoded per spec)
B, T, S = 32, 1, 8192
D, N, K, H = 3584, 28, 4, 128
TP, FS = 4, 2                # kv-head tensor-parallel x batch data-parallel
G = N // K                   # 7 q heads per kv head
BPC = B // FS                # 16 batches per core
DA = D + 128                 # contraction augmented with a bias row, padded
NDC = DA // 128              # 29 chunks of 128 along the contraction dim
ROPE_THETA = 1000000.0
NEG = -1.0e38
SCALE = float(H) ** -0.5
F32 = mybir.dt.float32
EXP = mybir.ActivationFunctionType.Exp

_GRAPH_CACHE: dict = {}


def _build(n_chunks: int, cstar: int, jslot: int, runs: tuple):
    """Build the per-core Bass graph (identical across all 8 cores)."""
    nc = bacc.Bacc(
        "TRN2", target_bir_lowering=False, debug=False,
        enable_asserts=False, num_devices=TP * FS,
    )
    L = n_chunks * 128
    kt_d = nc.dram_tensor("kt", [BPC, H, L], F32, kind="ExternalInput").ap()
    v_d = nc.dram_tensor("v", [BPC, H, L], F32, kind="ExternalInput").ap()
    xt_d = nc.dram_tensor("xt", [H, NDC * BPC], F32, kind="ExternalInput").ap()
    wq_d = nc.dram_tensor("wq", [G, H, DA], F32, kind="ExternalInput").ap()
    wk_d = nc.dram_tensor("wk", [H, DA], F32, kind="ExternalInput").ap()
    wv_d = nc.dram_tensor("wv", [H, DA], F32, kind="ExternalInput").ap()
    wo_d = nc.dram_tensor("wo", [G, H, D], F32, kind="ExternalInput").ap()
    cos_d = nc.dram_tensor("cos", [BPC, H // 2], F32, kind="ExternalInput").ap()
    sin_d = nc.dram_tensor("sin", [BPC, H // 2], F32, kind="ExternalInput").ap()
    mb_d = nc.dram_tensor("mb", [BPC, H, n_chunks], F32, kind="ExternalInput").ap()
    out_d = nc.dram_tensor("out", [BPC, D], F32, kind="ExternalOutput").ap()

    with tile.TileContext(nc) as tc:
        with (
            tc.tile_pool(name="const", bufs=1) as constp,
            tc.tile_pool(name="proj", bufs=2) as projp,
            tc.tile_pool(name="kv", bufs=2) as kvp,
            tc.tile_pool(name="small", bufs=2) as smallp,
            tc.tile_pool(name="wop", bufs=3) as wop,
            tc.tile_pool(name="ps_sst", bufs=2, space="PSUM") as ps_sst,
            tc.tile_pool(name="ps_o", bufs=2, space="PSUM") as ps_o,
            tc.tile_pool(name="ps_den", bufs=1, space="PSUM") as ps_den,
            tc.tile_pool(name="ps_misc", bufs=2, space="PSUM") as ps_misc,
        ):
            ident16 = constp.tile([16, 16], F32, tag="id16")
            make_identity(nc, ident16[:])
            ones_col = constp.tile([H, 1], F32, tag="onec")
            nc.vector.memset(ones_col[:], 1.0)
            ones_row = constp.tile([1, H], F32, tag="oner")
            nc.vector.memset(ones_row[:], 1.0)

            xt_sb = constp.tile([H, NDC * BPC], F32, tag="xt")
            nc.sync.dma_start(xt_sb[:], xt_d)
            cos_sb = constp.tile([BPC, H // 2], F32, tag="cos")
            nc.sync.dma_start(cos_sb[:], cos_d)
            sin_sb = constp.tile([BPC, H // 2], F32, tag="sin")
            nc.sync.dma_start(sin_sb[:], sin_d)

            qt_all = constp.tile([H, G * BPC], F32, tag="qt")   # col q*16+b
            k_newt = constp.tile([H, BPC], F32, tag="knt")
            v_new = constp.tile([BPC, H], F32, tag="vn")
            qkvt = constp.tile([H, BPC * G], F32, tag="qkvt")   # col b*7+q
            out_sb = constp.tile([BPC, D], F32, tag="osb")

            # ---- projections: 7 q heads, then k, then v ----
            for hh in range(G + 2):
                w_sb = projp.tile([H, DA], F32, tag="w")
                if hh < G:
                    nc.sync.dma_start(w_sb[:], wq_d[hh])
                elif hh == G:
                    nc.sync.dma_start(w_sb[:], wk_d)
                else:
                    nc.sync.dma_start(w_sb[:], wv_d)
                pp = ps_misc.tile([BPC, H], F32, tag="m")
                for c in range(NDC):
                    nc.tensor.matmul(
                        pp[:],
                        xt_sb[:, c * BPC:(c + 1) * BPC],
                        w_sb[:, c * 128:(c + 1) * 128],
                        start=(c == 0), stop=(c == NDC - 1),
                    )
                if hh == G + 1:
                    nc.scalar.copy(v_new[:], pp[:])
                    continue
                # RoPE on [16b, 128h]
                ro = projp.tile([BPC, H], F32, tag="ro")
                t1 = projp.tile([BPC, H // 2], F32, tag="t1")
                t2 = projp.tile([BPC, H // 2], F32, tag="t2")
                nc.vector.tensor_mul(t1[:], pp[:, 0:64], cos_sb[:])
                nc.vector.tensor_mul(t2[:], pp[:, 64:128], sin_sb[:])
                nc.vector.tensor_sub(ro[:, 0:64], t1[:], t2[:])
                nc.vector.tensor_mul(t1[:], pp[:, 64:128], cos_sb[:])
                nc.vector.tensor_mul(t2[:], pp[:, 0:64], sin_sb[:])
                nc.vector.tensor_add(ro[:, 64:128], t1[:], t2[:])
                pt = ps_misc.tile([H, BPC], F32, tag="m")
                nc.tensor.transpose(pt[:], ro[:], ident16[:])
                if hh < G:
                    nc.scalar.copy(qt_all[:, hh * BPC:(hh + 1) * BPC], pt[:])
                else:
                    nc.scalar.copy(k_newt[:], pt[:])

            qt_v = qt_all[:].rearrange("p (q b) -> p b q", b=BPC)

            # ---- attention over the cache, one batch at a time ----
            for b in range(BPC):
                kt_sb = kvp.tile([H, L], F32, tag="kt")
                nc.sync.dma_start(kt_sb[:], kt_d[b])
                v_sb = kvp.tile([H, L], F32, tag="v")
                nc.sync.dma_start(v_sb[:], v_d[b])
                mb_sb = smallp.tile([H, n_chunks], F32, tag="mb")
                nc.sync.dma_start(mb_sb[:], mb_d[b])

                # ring-buffer insert of the new token's k/v at slot idx
                col = cstar * 128 + jslot
                nc.scalar.copy(kt_sb[:, col:col + 1], k_newt[:, b:b + 1])
                nc.sync.dma_start(
                    out=v_sb[jslot:jslot + 1, cstar * 128:(cstar + 1) * 128],
                    in_=v_new[b:b + 1, :],
                )

                ptile = smallp.tile([H, n_chunks * G], F32, tag="pt")
                qt_b = qt_v[:, b, :]
                for (c0, ln) in runs[b]:
                    sst = ps_sst.tile([H, ln * G], F32, tag="sst")
                    for i in range(ln):
                        c = c0 + i
                        nc.tensor.matmul(
                            sst[:, i * G:(i + 1) * G],
                            kt_sb[:, c * 128:(c + 1) * 128],
                            qt_b,
                            start=True, stop=True,
                        )
                    nc.scalar.activation(
                        ptile[:, c0 * G:(c0 + ln) * G], sst[:],
                        EXP, bias=mb_sb[:, c0:c0 + 1], scale=SCALE,
                    )

                po = ps_o.tile([H, G], F32, tag="o")
                for c in range(n_chunks):
                    nc.tensor.matmul(
                        po[:],
                        v_sb[:, c * 128:(c + 1) * 128],
                        ptile[:, c * G:(c + 1) * G],
                        start=(c == 0), stop=(c == n_chunks - 1),
                    )
                pd = ps_den.tile([1, n_chunks * G], F32, tag="d")
                nc.tensor.matmul(pd[:], ones_col[:], ptile[:], start=True, stop=True)
                den = smallp.tile([1, G], F32, tag="den")
                nc.vector.tensor_reduce(
                    den[:], pd[:].rearrange("p (c q) -> p q c", q=G),
                    axis=mybir.AxisListType.X, op=mybir.AluOpType.add,
                )
                rec = smallp.tile([1, G], F32, tag="rec")
                nc.vector.reciprocal(rec[:], den[:])
                pb = ps_misc.tile([H, G], F32, tag="m")
                nc.tensor.matmul(pb[:], ones_row[:], rec[:], start=True, stop=True)
                bc = smallp.tile([H, G], F32, tag="bc")
                nc.scalar.copy(bc[:], pb[:])
                nc.vector.tensor_mul(qkvt[:, b * G:(b + 1) * G], po[:], bc[:])

            # ---- output projection: out[b, d] = sum_q qkvt[:,b,q].T @ wo[q] ----
            qkvt_v = qkvt[:].rearrange("p (b q) -> p q b", q=G)
            DT = D // 512
            for di in range(DT):
                ps = ps_misc.tile([BPC, 512], F32, tag="m")
                for q in range(G):
                    wo_sb = wop.tile([H, 512], F32, tag="wo")
                    nc.sync.dma_start(wo_sb[:], wo_d[q][:, di * 512:(di + 1) * 512])
                    nc.tensor.matmul(
                        ps[:], qkvt_v[:, q, :], wo_sb[:],
                        start=(q == 0), stop=(q == G - 1),
                    )
                nc.vector.tensor_copy(out_sb[:, di * 512:(di + 1) * 512], ps[:])
            nc.sync.dma_start(out_d, out_sb[:])

    nc.compile()
    return nc


def _prep(inputs):
    """Host-side: read runtime scalars, build per-core input maps."""
    x = np.asarray(inputs["x"], np.float32).reshape(B, D)
    seg = np.asarray(inputs["segment_pos"]).reshape(B).astype(np.float64)
    cache_k = np.asarray(inputs["cache_k"], np.float32)
    cache_v = np.asarray(inputs["cache_v"], np.float32)
    idx = int(np.asarray(inputs["end_index"]).reshape(-1)[0]) % S
    mask = np.asarray(inputs["attn_mask"]).reshape(B, S)
    wq = np.asarray(inputs["wq"], np.float32)
    wk = np.asarray(inputs["wk"], np.float32)
    wv = np.asarray(inputs["wv"], np.float32)
    wo = np.asarray(inputs["wo"], np.float32)
    q_bias = np.asarray(inputs["q_bias"], np.float32).reshape(N, H)
    k_bias = np.asarray(inputs["k_bias"], np.float32).reshape(K, H)
    v_bias = np.asarray(inputs["v_bias"], np.float32).reshape(K, H)

    true_idx = np.nonzero(mask.any(axis=0))[0]
    last = int(true_idx[-1]) if true_idx.size else 0
    L_eff = max(last + 1, idx + 1)
    n_chunks = (L_eff + 127) // 128
    L = n_chunks * 128
    assert L <= S
    cstar, jslot = idx // 128, idx % 128

    maskL = mask[:, :L]
    full_bc = maskL.reshape(B, n_chunks, 128).all(-1)
    full_slot = full_bc.reshape(FS, BPC, n_chunks).all(0)
    runs = []
    for b in range(BPC):
        rb, c = [], 0
        while c < n_chunks:
            if full_slot[b, c]:
                ln = 1
                while ln < 8 and c + ln < n_chunks and full_slot[b, c + ln]:
                    ln += 1
                rb.append((c, ln))
                c += ln
            else:
                rb.append((c, 1))
                c += 1
        runs.append(tuple(rb))
    runs = tuple(runs)

    # RoPE tables from actual positions
    frac = 2.0 * np.arange(H // 2, dtype=np.float64) / H
    timescale = ROPE_THETA ** frac
    sinu = seg[:, None] / timescale[None, :]
    cos_t = np.cos(sinu).astype(np.float32)
    sin_t = np.sin(sinu).astype(np.float32)

    # additive mask bias [B, 128, n_chunks]
    mb = np.where(maskL, 0.0, NEG).astype(np.float32)
    mb = mb.reshape(B, n_chunks, 128).transpose(0, 2, 1)

    in_maps = []
    for core in range(TP * FS):
        t, f = core // FS, core % FS
        bs = slice(f * BPC, (f + 1) * BPC)
        hs = slice(t * G, (t + 1) * G)

        kt = np.ascontiguousarray(
            cache_k[bs, :L, t, :].transpose(0, 2, 1))          # [16,128,L]
        vv = np.ascontiguousarray(
            cache_v[bs, :L, t, :].reshape(BPC, n_chunks, 128, H)
            .transpose(0, 2, 1, 3).reshape(BPC, H, L))         # [16,128,(c h)]

        x_aug = np.zeros((DA, BPC), np.float32)
        x_aug[:D] = x[bs].T
        x_aug[D] = 1.0
        xt = np.ascontiguousarray(
            x_aug.reshape(NDC, 128, BPC).transpose(1, 0, 2).reshape(H, NDC * BPC))

        wq_aug = np.zeros((DA, G, H), np.float32)
        wq_aug[:D] = wq[:, hs, :]
        wq_aug[D] = q_bias[hs]
        wq_c = np.ascontiguousarray(
            wq_aug.reshape(NDC, 128, G, H).transpose(2, 1, 0, 3).reshape(G, H, DA))

        def _aug_kv(w, bias_row):
            a = np.zeros((DA, H), np.float32)
            a[:D] = w[:, t, :]
            a[D] = bias_row
            return np.ascontiguousarray(
                a.reshape(NDC, 128, H).transpose(1, 0, 2).reshape(H, DA))

        wk_c = _aug_kv(wk, k_bias[t])
        wv_c = _aug_kv(wv, v_bias[t])
        wo_c = np.ascontiguousarray(wo[hs])

        in_maps.append({
            "kt": kt, "v": vv, "xt": xt,
            "wq": wq_c, "wk": wk_c, "wv": wv_c, "wo": wo_c,
            "cos": np.ascontiguousarray(cos_t[bs]),
            "sin": np.ascontiguousarray(sin_t[bs]),
            "mb": np.ascontiguousarray(mb[bs]),
        })
    return in_maps, (n_chunks, cstar, jslot, runs)


def _run(inputs, trace=False, tmpdir=None):
    in_maps, key = _prep(inputs)
    nc = _GRAPH_CACHE.get(key)
    if nc is None:
        nc = _build(*key)
        _GRAPH_CACHE[key] = nc
    res = run_bass_kernel_spmd(
        nc, in_maps, core_ids=list(range(TP * FS)),
        trace=trace, tmpdir=tmpdir,
    )
    out = np.zeros((B, T, D), np.float32)
    for core in range(TP * FS):
        f = core % FS
        out[f * BPC:(f + 1) * BPC, 0, :] += res.results[core]["out"]
    return out, res


def kernel(**inputs):
    out, _ = _run(inputs)
    return out


# revision 14
# speedup vs baseline: 1.3201x; 1.3201x over previous
"""Trainium2 Bass kernel: GQA decode attention (sparse / masked KV cache).

Module: q/k/v projections (+bias) -> RoPE(q, k_new) -> ring-buffer cache
insert at end_index -> masked softmax attention over the KV cache -> output
projection.

Sharding (8 NeuronCores): tp=4 over kv-heads (each tp rank owns 1 kv head and
its 7 GQA q heads, wq/wo/q_bias sharded on the head axis) x fsdp=2 over batch
(16 batches per rank) for cache/activations.  Each core computes a partial
output (its 7 heads' contribution to out[b, :]); the 4-way tp partial sum is
done host-side after gathering.

Device algorithm per core, per batch:
  scores^T[l, q] accumulated per 128-slot chunk via
      matmul(lhsT=KT_chunk[h,128l], rhs=QT_b[h,7])             (PSUM [128,7])
  P^T = exp(scale * scores^T + maskbias)                        (ScalarE, no
      max-subtraction: scores are bounded <<88 so fp32 exp is exact-safe)
  O^T[h, q]  += matmul(lhsT=V_chunk[l,h],  rhs=P^T_chunk[l,7])  (PSUM accum)
  denom[1,q] = matmul(lhsT=ones[l,1], rhs=P^T_all) + free-axis reduce
  QKV^T = O^T * broadcast(1/denom)
  out[b,:]  += matmul(lhsT=QKV^T[:, q, :], rhs=WO_q[h, d]) over q

K arrives host-pre-transposed per (batch, kv-head) as [h, l] so scores need no
on-device transpose; V stays in natural [l, h] chunk layout.  The new-token
k/v are projected on device, RoPE'd, and inserted into the SBUF-resident
chunk that holds slot end_index before use.
"""

import numpy as np

import concourse.bass as bass
import concourse.mybir as mybir
import concourse.tile as tile
from concourse import bacc
from concourse.bass_utils import run_bass_kernel_spmd
from concourse.masks import make_identity

# Problem shapes (hardcoded per spec)
B, T, S = 32, 1, 8192
D, N, K, H = 3584, 28, 4, 128
TP, FS = 4, 2                # kv-head tensor-parallel x batch data-parallel
G = N // K                   # 7 q heads per kv head
BPC = B // FS                # 16 batches per core
DA = D + 128                 # contraction augmented with a bias row, padded
NDC = DA // 128              # 29 chunks of 128 along the contraction dim
ROPE_THETA = 1000000.0
NEG = -1.0e38
SCALE = float(H) ** -0.5
F32 = mybir.dt.float32
F32R = mybir.dt.float32r
EXP = mybir.ActivationFunctionType.Exp
NH = G + 2                   # 9 projection heads per core: 7 q, 1 k, 1 v
GP = 8                       # G padded to even for fp32r ISA restrictions


_GRAPH_CACHE: dict = {}


def _build(n_chunks: int, cstar: int, jslot: int, runs: tuple):
    """Build the per-core Bass graph (identical across all 8 cores)."""
    nc = bacc.Bacc(
        "TRN2", target_bir_lowering=False, debug=False,
        enable_asserts=False, num_devices=TP * FS,
    )
    L = n_chunks * 128
    kt_d = nc.dram_tensor("kt", [BPC, H, L], F32R, kind="ExternalInput").ap()
    v_d = nc.dram_tensor("v", [BPC, H, L], F32R, kind="ExternalInput").ap()
    xt_d = nc.dram_tensor("xt", [H, NDC * BPC], F32R, kind="ExternalInput").ap()
    # all 9 heads' projection weights, chunked: [c][d-in-chunk][head*128+h]
    wp_d = nc.dram_tensor("wp", [NDC, H, NH * H], F32R, kind="ExternalInput").ap()
    wo_d = nc.dram_tensor("wo", [G, H, D], F32R, kind="ExternalInput").ap()
    cos_d = nc.dram_tensor("cos", [BPC, H // 2], F32, kind="ExternalInput").ap()
    sin_d = nc.dram_tensor("sin", [BPC, H // 2], F32, kind="ExternalInput").ap()
    mb_d = nc.dram_tensor("mb", [BPC, H, n_chunks], F32, kind="ExternalInput").ap()
    out_d = nc.dram_tensor("out", [BPC, D], F32, kind="ExternalOutput").ap()

    with tile.TileContext(nc) as tc:
        with (
            tc.tile_pool(name="const", bufs=1) as constp,
            tc.tile_pool(name="proj", bufs=2) as projp,
            tc.tile_pool(name="kv", bufs=2) as kvp,
            tc.tile_pool(name="small", bufs=2) as smallp,
            tc.tile_pool(name="wop", bufs=3) as wop,
            tc.tile_pool(name="ps_sst", bufs=2, space="PSUM") as ps_sst,
            tc.tile_pool(name="ps_o", bufs=2, space="PSUM") as ps_o,
            tc.tile_pool(name="ps_den", bufs=1, space="PSUM") as ps_den,
            tc.tile_pool(name="ps_misc", bufs=2, space="PSUM") as ps_misc,
        ):
            ident16 = constp.tile([16, 16], F32, tag="id16")
            make_identity(nc, ident16[:])
            ones_f32 = constp.tile([H, 2], F32, tag="onef")
            nc.vector.memset(ones_f32[:], 1.0)
            ones_col = constp.tile([H, 2], F32R, tag="onec")
            nc.scalar.copy(ones_col[:], ones_f32[:])
            ones_row = constp.tile([1, H], F32, tag="oner")
            nc.vector.memset(ones_row[:], 1.0)
            zero_f32 = constp.tile([H, BPC], F32, tag="zf")
            nc.vector.memset(zero_f32[:], 0.0)

            xt_sb = constp.tile([H, NDC * BPC], F32R, tag="xt")
            nc.sync.dma_start(xt_sb[:], xt_d)
            cos_sb = constp.tile([BPC, H // 2], F32, tag="cos")
            nc.sync.dma_start(cos_sb[:], cos_d)
            sin_sb = constp.tile([BPC, H // 2], F32, tag="sin")
            nc.sync.dma_start(sin_sb[:], sin_d)

            qt_all = constp.tile([H, GP * BPC], F32R, tag="qt")  # col q*16+b
            nc.scalar.copy(qt_all[:, G * BPC:], zero_f32[:])  # fp32r pad head
            k_newt = constp.tile([H, BPC], F32R, tag="knt")
            v_new = constp.tile([BPC, H], F32R, tag="vn")
            qkvt = constp.tile([H, BPC * G], F32R, tag="qkvt")   # col b*7+q
            out_sb = constp.tile([BPC, D], F32, tag="osb")

            # ---- projections: 9 heads in 512-col groups (fp32r fast path) ----
            # wp columns: q0..q6 | k | v at head*128.  Groups: [0:512]=q0-3,
            # [512:1024]=q4-6+k, [1024:1152]=v.
            groups = [(0, 512), (512, 512), (1024, NH * H - 1024)]

            def _rope_head(pp, pcol, hh):
                """RoPE psum slice [16,128] at pcol, transpose, store."""
                ro = projp.tile([BPC, H], F32, tag="ro")
                t1 = projp.tile([BPC, H // 2], F32, tag="t1")
                t2 = projp.tile([BPC, H // 2], F32, tag="t2")
                nc.vector.tensor_mul(t1[:], pp[:, pcol:pcol + 64], cos_sb[:])
                nc.vector.tensor_mul(t2[:], pp[:, pcol + 64:pcol + 128], sin_sb[:])
                nc.vector.tensor_sub(ro[:, 0:64], t1[:], t2[:])
                nc.vector.tensor_mul(t1[:], pp[:, pcol + 64:pcol + 128], cos_sb[:])
                nc.vector.tensor_mul(t2[:], pp[:, pcol:pcol + 64], sin_sb[:])
                nc.vector.tensor_add(ro[:, 64:128], t1[:], t2[:])
                pt = ps_misc.tile([H, BPC], F32, tag="m")
                nc.tensor.transpose(pt[:], ro[:], ident16[:])
                if hh < G:
                    nc.scalar.copy(qt_all[:, hh * BPC:(hh + 1) * BPC], pt[:])
                else:
                    nc.scalar.copy(k_newt[:], pt[:])

            for (g0, gw) in groups:
                pp = ps_misc.tile([BPC, 512], F32, tag="m")
                for c in range(NDC):
                    w_sb = projp.tile([H, 512], F32R, tag="w")
                    nc.sync.dma_start(w_sb[:, 0:gw], wp_d[c][:, g0:g0 + gw])
                    nc.tensor.matmul(
                        pp[:, 0:gw],
                        xt_sb[:, c * BPC:(c + 1) * BPC],
                        w_sb[:, 0:gw],
                        start=(c == 0), stop=(c == NDC - 1),
                    )
                for hh_loc in range(gw // H):
                    hh = (g0 // H) + hh_loc
                    if hh == G + 1:
                        nc.scalar.copy(v_new[:], pp[:, hh_loc * H:(hh_loc + 1) * H])
                    else:
                        _rope_head(pp, hh_loc * H, hh)

            qt_v = qt_all[:].rearrange("p (q b) -> p b q", b=BPC)  # q dim = GP

            # ---- attention over the cache, one batch at a time ----
            for b in range(BPC):
                kt_sb = kvp.tile([H, L], F32R, tag="kt")
                nc.sync.dma_start(kt_sb[:], kt_d[b])
                v_sb = kvp.tile([H, L], F32R, tag="v")
                nc.sync.dma_start(v_sb[:], v_d[b])
                mb_sb = smallp.tile([H, n_chunks], F32, tag="mb")
                nc.sync.dma_start(mb_sb[:], mb_d[b])

                # ring-buffer insert of the new token's k/v at slot idx
                col = cstar * 128 + jslot
                nc.scalar.copy(kt_sb[:, col:col + 1], k_newt[:, b:b + 1])
                nc.sync.dma_start(
                    out=v_sb[jslot:jslot + 1, cstar * 128:(cstar + 1) * 128],
                    in_=v_new[b:b + 1, :],
                )

                ptile = smallp.tile([H, n_chunks * GP], F32R, tag="pt")
                qt_b = qt_v[:, b, :]
                for (c0, ln) in runs[b]:
                    sst = ps_sst.tile([H, ln * GP], F32, tag="sst")
                    for i in range(ln):
                        c = c0 + i
                        nc.tensor.matmul(
                            sst[:, i * GP:(i + 1) * GP],
                            kt_sb[:, c * 128:(c + 1) * 128],
                            qt_b,
                            start=True, stop=True,
                        )
                    nc.scalar.activation(
                        ptile[:, c0 * GP:(c0 + ln) * GP], sst[:],
                        EXP, bias=mb_sb[:, c0:c0 + 1], scale=SCALE,
                    )

                po = ps_o.tile([H, GP], F32, tag="o")
                for c in range(n_chunks):
                    nc.tensor.matmul(
                        po[:],
                        v_sb[:, c * 128:(c + 1) * 128],
                        ptile[:, c * GP:(c + 1) * GP],
                        start=(c == 0), stop=(c == n_chunks - 1),
                    )
                pd = ps_den.tile([2, n_chunks * GP], F32, tag="d")
                nc.tensor.matmul(pd[:], ones_col[:], ptile[:],
                                 start=True, stop=True)
                den = smallp.tile([1, GP], F32, tag="den")
                nc.vector.tensor_reduce(
                    den[:], pd[0:1].rearrange("p (c q) -> p q c", q=GP),
                    axis=mybir.AxisListType.X, op=mybir.AluOpType.add,
                )
                rec = smallp.tile([1, GP], F32, tag="rec")
                nc.vector.reciprocal(rec[:], den[:])
                pb = ps_misc.tile([H, GP], F32, tag="m")
                nc.tensor.matmul(pb[:], ones_row[:], rec[:], start=True, stop=True)
                bc = smallp.tile([H, GP], F32, tag="bc")
                nc.scalar.copy(bc[:], pb[:])
                nc.vector.tensor_mul(qkvt[:, b * G:(b + 1) * G],
                                     po[:, 0:G], bc[:, 0:G])

            # ---- output projection: out[b, d] = sum_q qkvt[:,b,q].T @ wo[q] ----
            qkvt_v = qkvt[:].rearrange("p (b q) -> p q b", q=G)
            DT = D // 512
            for di in range(DT):
                ps = ps_misc.tile([BPC, 512], F32, tag="m")
                for q in range(G):
                    wo_sb = wop.tile([H, 512], F32R, tag="wo")
                    nc.sync.dma_start(wo_sb[:], wo_d[q][:, di * 512:(di + 1) * 512])
                    nc.tensor.matmul(
                        ps[:], qkvt_v[:, q, :], wo_sb[:],
                        start=(q == 0), stop=(q == G - 1),
                    )
                nc.vector.tensor_copy(out_sb[:, di * 512:(di + 1) * 512], ps[:])
            nc.sync.dma_start(out_d, out_sb[:])

    nc.compile()
    return nc


def _prep(inputs):
    """Host-side: read runtime scalars, build per-core input maps."""
    x = np.asarray(inputs["x"], np.float32).reshape(B, D)
    seg = np.asarray(inputs["segment_pos"]).reshape(B).astype(np.float64)
    cache_k = np.asarray(inputs["cache_k"], np.float32)
    cache_v = np.asarray(inputs["cache_v"], np.float32)
    idx = int(np.asarray(inputs["end_index"]).reshape(-1)[0]) % S
    mask = np.asarray(inputs["attn_mask"]).reshape(B, S)
    wq = np.asarray(inputs["wq"], np.float32)
    wk = np.asarray(inputs["wk"], np.float32)
    wv = np.asarray(inputs["wv"], np.float32)
    wo = np.asarray(inputs["wo"], np.float32)
    q_bias = np.asarray(inputs["q_bias"], np.float32).reshape(N, H)
    k_bias = np.asarray(inputs["k_bias"], np.float32).reshape(K, H)
    v_bias = np.asarray(inputs["v_bias"], np.float32).reshape(K, H)

    true_idx = np.nonzero(mask.any(axis=0))[0]
    last = int(true_idx[-1]) if true_idx.size else 0
    L_eff = max(last + 1, idx + 1)
    n_chunks = (L_eff + 127) // 128
    L = n_chunks * 128
    assert L <= S
    cstar, jslot = idx // 128, idx % 128

    maskL = mask[:, :L]
    full_bc = maskL.reshape(B, n_chunks, 128).all(-1)
    full_slot = full_bc.reshape(FS, BPC, n_chunks).all(0)
    runs = []
    for b in range(BPC):
        rb, c = [], 0
        while c < n_chunks:
            if full_slot[b, c]:
                ln = 1
                while ln < 8 and c + ln < n_chunks and full_slot[b, c + ln]:
                    ln += 1
                rb.append((c, ln))
                c += ln
            else:
                rb.append((c, 1))
                c += 1
        runs.append(tuple(rb))
    runs = tuple(runs)

    # RoPE tables from actual positions
    frac = 2.0 * np.arange(H // 2, dtype=np.float64) / H
    timescale = ROPE_THETA ** frac
    sinu = seg[:, None] / timescale[None, :]
    cos_t = np.cos(sinu).astype(np.float32)
    sin_t = np.sin(sinu).astype(np.float32)

    # additive mask bias [B, 128, n_chunks]
    mb = np.where(maskL, 0.0, NEG).astype(np.float32)
    mb = mb.reshape(B, n_chunks, 128).transpose(0, 2, 1)

    in_maps = []
    for core in range(TP * FS):
        t, f = core // FS, core % FS
        bs = slice(f * BPC, (f + 1) * BPC)
        hs = slice(t * G, (t + 1) * G)

        kt = np.ascontiguousarray(
            cache_k[bs, :L, t, :].transpose(0, 2, 1))          # [16,128,L]
        vv = np.ascontiguousarray(
            cache_v[bs, :L, t, :].reshape(BPC, n_chunks, 128, H)
            .transpose(0, 2, 1, 3).reshape(BPC, H, L))         # [16,128,(c h)]

        x_aug = np.zeros((DA, BPC), np.float32)
        x_aug[:D] = x[bs].T
        x_aug[D] = 1.0
        xt = np.ascontiguousarray(
            x_aug.reshape(NDC, 128, BPC).transpose(1, 0, 2).reshape(H, NDC * BPC))

        # combined projection weights [DA, 9 heads, H]: q0..q6, k, v (+bias row)
        wp_aug = np.zeros((DA, NH, H), np.float32)
        wp_aug[:D, :G] = wq[:, hs, :]
        wp_aug[:D, G] = wk[:, t, :]
        wp_aug[:D, G + 1] = wv[:, t, :]
        wp_aug[D, :G] = q_bias[hs]
        wp_aug[D, G] = k_bias[t]
        wp_aug[D, G + 1] = v_bias[t]
        wp_c = np.ascontiguousarray(
            wp_aug.reshape(NDC, 128, NH * H))
        wo_c = np.ascontiguousarray(wo[hs])

        in_maps.append({
            "kt": kt, "v": vv, "xt": xt,
            "wp": wp_c, "wo": wo_c,
            "cos": np.ascontiguousarray(cos_t[bs]),
            "sin": np.ascontiguousarray(sin_t[bs]),
            "mb": np.ascontiguousarray(mb[bs]),
        })
    return in_maps, (n_chunks, cstar, jslot, runs)


def _run(inputs, trace=False, tmpdir=None):
    in_maps, key = _prep(inputs)
    nc = _GRAPH_CACHE.get(key)
    if nc is None:
        nc = _build(*key)
        _GRAPH_CACHE[key] = nc
    res = run_bass_kernel_spmd(
        nc, in_maps, core_ids=list(range(TP * FS)),
        trace=trace, tmpdir=tmpdir,
    )
    out = np.zeros((B, T, D), np.float32)
    for core in range(TP * FS):
        f = core % FS
        out[f * BPC:(f + 1) * BPC, 0, :] += res.results[core]["out"]
    return out, res


def kernel(**inputs):
    out, _ = _run(inputs)
    return out
